# revision 1
# baseline (speedup 1.0000x reference)
"""W8A8 merged linear (nn_MergedW8A8Linear) on 8 TRN2 NeuronCores.

Column-parallel: weight/scale/bias sharded along out_features (1280/core),
x replicated. Per core:

  - activations: dynamic per-token int8 quant done on-device (absmax ->
    reciprocal -> round-to-int trick), operating on host-relayouted xT.
  - weights: streamed as raw int8 bytes (w+128 in [1,255]); converted
    on-device to EXACT fp16 values 1 + b/1024 by bit-twiddling uint16/uint32
    views on the DVE (fp16 bits = 0x3C00 | b).  The matmul then computes
    mm = sum_k xq * (1 + (w+128)/1024) in fp32-exact arithmetic, and the
    integer GEMM is recovered as  acc = 1024*mm - 1152*rowsum(xq), with
    rowsum taken exactly from spare columns encoded as 1.0 (byte 0).
  - byte-pair split: u16 low bytes -> "ev" half, high bytes -> "od" half;
    device output columns are [ev | od] interleave-permuted; the host
    inverse-permutes at the end.
  - dequant fused on-device: out = (1024*mm - 1152*rs)*xs[m]*ws[n] + b[n]
    via ACT per-partition scale/bias + DVE tensor_tensor with fp16 tiles.
"""
import contextlib
import numpy as np
import ml_dtypes

from concourse import bacc, tile, mybir
from concourse.bass_utils import run_bass_kernel_spmd

M = 64
K = 8192
KT = K // 128          # 64 k-tiles
N_TOTAL = 10240
NCORES = 8
NS = N_TOTAL // NCORES  # 1280 weight cols per core
NB = NS + 4             # bytes per row incl 4 rs cols (div by 4)
NU = NB // 2            # 642 u16 per row; ev cols = NU, od cols = NU
G = 4                   # k-tiles per DMA/convert group
NG = KT // G            # 16 groups
PSPLITS = [(0, 512), (512, 512), (1024, NB - 1024)]  # matmul n-slices
DQSPLITS = [(0, 1024), (1024, NB - 1024)]      # dequant slices (acc01, acc2)
RS_EVCOL = NS // 2      # ev index of byte col NS (rs col, byte 0 -> 1.0) = 640
RC = 12582912.0         # 1.5*2**23 round-to-nearest-int constant

f16 = mybir.dt.float16
f32 = mybir.dt.float32
u16 = mybir.dt.uint16
u32 = mybir.dt.uint32
i8 = mybir.dt.int8

_CACHE = {}


def build(repeats=1, hw_loop=0, probe="full", mult_eng="gpsimd"):
    """probe: 'p1' = weight DMA only; 'p2' = +conversion; 'p3' = +matmul;
    'p4' = +quant (no dequant); 'full' = everything."""
    do_conv = probe in ("p2", "p3", "p4", "full")
    do_mm = probe in ("p3", "p4", "full")
    do_quant = probe in ("p4", "full")
    do_deq = probe == "full"

    nc = bacc.Bacc("TRN2", target_bir_lowering=False, debug=False,
                   num_devices=NCORES)
    xT_d = nc.dram_tensor("xT", [128, KT, M], f16, kind="ExternalInput")
    wb_d = nc.dram_tensor("wb", [128, KT, NB], i8, kind="ExternalInput")
    ident_d = nc.dram_tensor("ident", [128, 128], f32, kind="ExternalInput")
    ident16_d = nc.dram_tensor("ident16", [128, 128], f16, kind="ExternalInput")
    wsb_d = nc.dram_tensor("wsb", [M, NB], f16, kind="ExternalInput")
    bb_d = nc.dram_tensor("bb", [M, NB], f16, kind="ExternalInput")
    out_d = nc.dram_tensor("out", [M, NB], f16, kind="ExternalOutput")
    dbg_d = nc.dram_tensor("dbg", [128, NG], i8, kind="ExternalOutput")

    with tile.TileContext(nc) as tc:
        with (
            tc.tile_pool(name="cst", bufs=1) as cst,
            tc.tile_pool(name="qp", bufs=1) as qp,
            tc.tile_pool(name="wp", bufs=8) as wp,
            tc.tile_pool(name="fp", bufs=6) as fp,
            tc.tile_pool(name="op", bufs=1) as op,
            tc.tile_pool(name="ps", bufs=1, space="PSUM") as ps,
            tc.tile_pool(name="psq", bufs=1, space="PSUM") as psq,
        ):
            ident = cst.tile([128, 128], f32, tag="ident")
            nc.sync.dma_start(out=ident[:], in_=ident_d[:])
            ident16 = cst.tile([128, 128], f16, tag="ident16")
            nc.sync.dma_start(out=ident16[:], in_=ident16_d[:])
            wsb = cst.tile([M, NB], f16, tag="wsb")
            nc.sync.dma_start(out=wsb[:], in_=wsb_d[:])
            bb = cst.tile([M, NB], f16, tag="bb")
            nc.sync.dma_start(out=bb[:], in_=bb_d[:])
            ones = cst.tile([1, 128], f32, tag="ones")
            nc.gpsimd.memset(ones[:], 1.0)
            rc_p = cst.tile([128, 1], f32, tag="rc_p")
            nc.gpsimd.memset(rc_p[:], RC)
            rc_n = cst.tile([128, 1], f32, tag="rc_n")
            nc.gpsimd.memset(rc_n[:], -RC)
            xq_c = cst.tile([128, 2, M], f16, tag="xq_c")
            nc.gpsimd.memset(xq_c[:], 1.0)
            wf_c = cst.tile([128, 2, NU], u16, tag="wf_c")
            nc.gpsimd.memset(wf_c[:], 15360)
            dbg = cst.tile([128, NG], i8, tag="dbg")
            nc.gpsimd.memset(dbg[:], 0)

            loop_cm = tc.For_i(0, hw_loop, 1) if hw_loop else contextlib.nullcontext()
            with loop_cm:
              for _ in range(repeats):
                # ---------------- quant pipeline ----------------
                xts = qp.tile([128, KT, M], f16, tag="xts")
                nc.scalar.dma_start(out=xts[:], in_=xT_d[:])
                xq = qp.tile([128, KT, M], f16, tag="xq")
                if do_quant:
                    xabs = qp.tile([128, KT, M], f16, tag="xabs")
                    nc.vector.tensor_scalar(
                        xabs[:].bitcast(u16), xts[:].bitcast(u16), 0x7FFF, None,
                        op0=mybir.AluOpType.bitwise_and,
                        op1=mybir.AluOpType.bypass)
                    h = KT // 2
                    while h >= 8:
                        nc.vector.tensor_tensor(
                            xabs[:, 0:h, :], xabs[:, 0:h, :], xabs[:, h:2 * h, :],
                            mybir.AluOpType.max)
                        h //= 2
                    pm = qp.tile([128, M], f16, tag="pm")
                    nc.vector.tensor_reduce(
                        pm[:], xabs[:, 0:8, :].rearrange("p t m -> p m t"),
                        axis=mybir.AxisListType.X, op=mybir.AluOpType.max)
                    pmT_ps = psq.tile([M, 128], f16, tag="pmT")
                    nc.tensor.transpose(pmT_ps[:], pm[:], ident16[:])
                    am = qp.tile([M, 1], f16, tag="am")
                    nc.vector.tensor_reduce(am[:], pmT_ps[:],
                                            axis=mybir.AxisListType.X,
                                            op=mybir.AluOpType.max)
                    xs = qp.tile([M, 1], f32, tag="xs")
                    nc.scalar.mul(xs[:], am[:], 1.0 / 127.0)
                    inv = qp.tile([M, 1], f32, tag="inv")
                    nc.vector.reciprocal(inv[:], xs[:])
                    sxs = qp.tile([M, 1], f32, tag="sxs")
                    nc.scalar.mul(sxs[:], xs[:], 1024.0)
                    invT_ps = psq.tile([1, M], f32, tag="invT")
                    nc.tensor.transpose(invT_ps[:], inv[:], ident[0:M, 0:M])
                    inv_row = qp.tile([1, M], f32, tag="inv_row")
                    nc.scalar.copy(inv_row[:], invT_ps[:])
                    invb_ps = psq.tile([128, M], f32, tag="invb")
                    nc.tensor.matmul(invb_ps[:], ones[:], inv_row[:],
                                     start=True, stop=True)
                    invb = qp.tile([128, M], f32, tag="invb_sb")
                    nc.vector.tensor_copy(invb[:], invb_ps[:])

                    QC = 4                     # quant chunks
                    CK = KT // QC
                    invb_rep = invb[:].unsqueeze(1).broadcast_to([128, CK, M])
                    me = nc.gpsimd if mult_eng == "gpsimd" else nc.vector
                    xr = qp.tile([128, CK, M], f32, tag="xr")
                    for q in range(QC):
                        xq32_q = qp.tile([128, CK, M], f32, tag="xq32")
                        mq = nc.gpsimd if (mult_eng == "gpsimd" or
                                           (mult_eng == "mix" and q % 2 == 1)) else nc.vector
                        mq.tensor_tensor(xq32_q[:], xts[:, q * CK:(q + 1) * CK, :],
                                         invb_rep, mybir.AluOpType.mult)
                        nc.scalar.activation(xr[:], xq32_q[:],
                                             mybir.ActivationFunctionType.Identity,
                                             bias=rc_p[:], scale=1.0)
                        nc.scalar.activation(xq[:, q * CK:(q + 1) * CK, :], xr[:],
                                             mybir.ActivationFunctionType.Identity,
                                             bias=rc_n[:], scale=1.0)

                # -------- weight stream: dma + convert + mm --------
                if do_mm:
                    acc01 = ps.tile([128, 1024], f32, tag="acc01", name="acc01")
                    acc2 = ps.tile([128, NB - 1024], f32, tag="acc2", name="acc2")
                    accv = [acc01[:, 0:512], acc01[:, 512:1024], acc2[:]]
                    dq = [acc01, acc2]
                for g in range(NG):
                    wraw = wp.tile([128, G, NB], i8, tag="wraw")
                    eng = nc.sync if g % 2 == 0 else nc.scalar
                    eng.dma_start(
                        out=wraw[:],
                        in_=wb_d[:, g * G:(g + 1) * G, :])
                    if do_conv:
                        wf = fp.tile([128, G, 2, NU], u16, tag="wf")
                        nc.vector.tensor_scalar(
                            wf[:, :, 0, :], wraw[:].bitcast(u16),
                            0x00FF, 0x3C00,
                            op0=mybir.AluOpType.bitwise_and,
                            op1=mybir.AluOpType.bitwise_or)
                        nc.vector.tensor_scalar(
                            wf[:, :, 1, :], wraw[:].bitcast(u16),
                            8, 0x3C00,
                            op0=mybir.AluOpType.logical_shift_right,
                            op1=mybir.AluOpType.bitwise_or)
                    else:
                        nc.vector.tensor_copy(dbg[:, g:g + 1], wraw[:, 0, 0:1])
                    if do_mm:
                        for t in range(G):
                            kt = g * G + t
                            cg = kt % 2
                            if do_conv:
                                rhs = wf[:, t, :, :].rearrange("p a n -> p (a n)").bitcast(f16)
                            else:
                                rhs = wf_c[:].rearrange("p a n -> p (a n)").bitcast(f16)
                            lhsT = xq[:, kt, :] if do_quant else xq_c[:, kt % 2, :]
                            for j, (o, w) in enumerate(PSPLITS):
                                nc.tensor.matmul(
                                    accv[j][cg * 64:(cg + 1) * 64, :],
                                    lhsT, rhs[:, o:o + w],
                                    start=(kt < 2), stop=(kt >= KT - 2))
                    elif do_conv:
                        nc.vector.tensor_copy(dbg[:, g:g + 1],
                                              wf[:, 0, 0, 0:1].bitcast(i8)[:, 0:1])

                # ---------------- dequant ----------------
                if do_deq:
                    t1s = []
                    for j, (o, w) in enumerate(PSPLITS):
                        th = op.tile([M, 512], f32, tag=f"th_{j}", name=f"th_{j}")
                        nc.scalar.copy(th[:, 0:w], accv[j][64:128, :])
                        t1 = op.tile([M, 512], f32, tag=f"t1_{j}", name=f"t1_{j}")
                        nc.vector.tensor_tensor(t1[:, 0:w], accv[j][0:64, :],
                                                th[:, 0:w], mybir.AluOpType.add)
                        t1s.append(t1)
                    rs = t1s[1][:, RS_EVCOL - 512:RS_EVCOL - 511]
                    nrs = op.tile([M, 1], f32, tag="nrs")
                    nc.vector.tensor_tensor(nrs[:], rs, sxs[:], mybir.AluOpType.mult)
                    nc.scalar.mul(nrs[:], nrs[:], -1.125)
                    for j, (o, w) in enumerate(PSPLITS):
                        t2 = op.tile([M, 512], f32, tag=f"t2_{j}", name=f"t2_{j}")
                        nc.scalar.activation(t2[:, 0:w], t1s[j][:, 0:w],
                                             mybir.ActivationFunctionType.Identity,
                                             bias=nrs[:], scale=sxs[:])
                        t3 = op.tile([M, 512], f16, tag=f"t3_{j}", name=f"t3_{j}")
                        nc.vector.tensor_tensor(t3[:, 0:w], t2[:, 0:w],
                                                wsb[:, o:o + w], mybir.AluOpType.mult)
                        t4 = op.tile([M, 512], f16, tag=f"t4_{j}", name=f"t4_{j}")
                        nc.vector.tensor_tensor(t4[:, 0:w], t3[:, 0:w],
                                                bb[:, o:o + w], mybir.AluOpType.add)
                        nc.sync.dma_start(out=out_d[:, o:o + w], in_=t4[:, 0:w])
                elif do_mm:
                    t4 = op.tile([M, 512], f16, tag="t4_0", name="t4_0")
                    nc.vector.tensor_copy(t4[:, 0:512], acc01[0:64, 0:512])
                    nc.sync.dma_start(out=out_d[:, 0:512], in_=t4[:, 0:512])
            nc.sync.dma_start(out=dbg_d[:], in_=dbg[:])
    nc.compile()
    return nc


def _prep_inputs(x, weight, scale, bias):
    x = np.asarray(x)
    weight = np.asarray(weight)
    scale = np.asarray(scale, dtype=np.float32)
    bias = np.asarray(bias)
    if weight.dtype != np.int8:
        weight = weight.astype(np.int8)
    x16 = x.astype(np.float16, copy=False)
    # xT_dev[p, t, m] = x[m, t*128+p]
    xT_dev = np.ascontiguousarray(
        x16.T.reshape(KT, 128, M).transpose(1, 0, 2))

    # device column order: [ev bytes 0,2,..  | od bytes 1,3,..]
    ev = np.arange(0, NB, 2)
    od = np.arange(1, NB, 2)
    perm = np.concatenate([ev, od])           # device col j <- byte col perm[j]

    ident = np.eye(128, dtype=np.float32)
    ident16 = np.eye(128, dtype=np.float16)
    in_maps = []
    for c in range(NCORES):
        sl = slice(c * NS, (c + 1) * NS)
        wbytes = np.zeros((K, NB), dtype=np.uint8)
        wbytes[:, :NS] = (weight[sl, :].T.astype(np.int16) + 128).astype(np.uint8)
        wbytes = np.ascontiguousarray(wbytes.reshape(KT, 128, NB).transpose(1, 0, 2))
        ws_full = np.zeros((NB,), dtype=np.float32)
        ws_full[:NS] = scale[sl, 0]
        b_full = np.zeros((NB,), dtype=np.float32)
        b_full[:NS] = bias[sl].astype(np.float32)
        wsb = np.tile(ws_full[perm][None, :], (M, 1)).astype(np.float16)
        bb = np.tile(b_full[perm][None, :], (M, 1)).astype(np.float16)
        in_maps.append({
            "xT": xT_dev,
            "wb": wbytes.view(np.int8),
            "ident": ident,
            "ident16": ident16,
            "wsb": wsb,
            "bb": bb,
        })
    return in_maps, perm


def assemble_output(results, perm, out_dtype):
    inv_perm = np.argsort(perm)
    out = np.empty((M, N_TOTAL), dtype=np.float16)
    for c in range(NCORES):
        dev = results[c]["out"]                 # [M, NB] device (permuted cols)
        out[:, c * NS:(c + 1) * NS] = dev[:, inv_perm][:, :NS]
    return out.astype(out_dtype, copy=False)


def kernel(x, weight, scale, bias):
    in_maps, perm = _prep_inputs(x, weight, scale, bias)
    if "nc" not in _CACHE:
        _CACHE["nc"] = build()
    nc = _CACHE["nc"]
    res = run_bass_kernel_spmd(nc, in_maps, list(range(NCORES)))
    return assemble_output(res.results, perm, np.asarray(x).dtype)

